# revision 1
# baseline (speedup 1.0000x reference)
"""Trainium2 Bass kernel for a channel-attention block.

Per batch b (one NeuronCore each, 8 total):
    v      = x[b].reshape(C, H*W)                    # [256, 16384]
    energy = v @ v.T                                 # [256, 256]
    w      = softmax(max(energy, -1) - energy, -1)   # == softmax(-energy)
    y      = alpha * (w @ v) + x[b]

Layout / strategy (per core):
  - v stays resident in SBUF as [128, 2, 16384] f32r (c = h*128 + p).
  - energy needs s on partitions, so each 128-wide s-tile of v is
    transposed on the PE, then fed to two float32r matmuls (FP22
    truncation, full bf16-rate at N>=256) accumulating [128, 256] PSUM
    tiles over all 128 s-tiles.  The transpose+copy for k+1 is emitted
    one iteration ahead of k's matmuls (software pipeline) and the
    PSUM->SBUF copies alternate ScalarE/VectorE.
  - Stable softmax via one reduce-min + one fused ScalarE
    exp(-energy + rowmin) with accumulated row-sum, then reciprocal
    multiply.  (softmax(max-e) == exp(rowmin-e)/sum.)
  - w is PE-transposed to wT; second matmul contracts over channels with
    v in natural layout; alpha*psum + x fused on VectorE; 2 MB staged
    output DMAs.  PSUM pools are scoped per phase: B/C get 6 transpose
    banks, D gets 4 output banks.
"""

from contextlib import ExitStack

import numpy as np

import concourse.bass as bass
import concourse.mybir as mybir
import concourse.tile as tile
from concourse import bacc
from concourse.bass_utils import run_bass_kernel_spmd
from concourse.masks import make_identity

B, C, HH, WW = 8, 256, 128, 128
HW = HH * WW            # 16384
P = 128
H = C // P              # 2 channel chunks
KT = HW // P            # 128 contraction tiles for energy
S_CHUNK = 2048          # columns per streaming DMA chunk (2 MB)
N_CHUNKS = HW // S_CHUNK
S_TILE = 512            # second-matmul moving free dim (1 PSUM bank)
J_PER_CHUNK = S_CHUNK // S_TILE

F32 = mybir.dt.float32
F32R = mybir.dt.float32r


def emit(nc, tc, alpha, ident_r, v_sb, x_v, y_v):
    """One full per-core pass (phases A-D). Pools are scoped inside."""
    # ---- Phase A: stream x into SBUF.  The first chunk is split into
    # quarters so the PE can start transposing after ~512 KB instead of 2 MB.
    for q in range(4):
        sl = slice(q * (S_CHUNK // 4), (q + 1) * (S_CHUNK // 4))
        nc.sync.dma_start(out=v_sb[:, :, sl], in_=x_v[:, :, sl])
    for ck in range(1, N_CHUNKS):
        sl = slice(ck * S_CHUNK, (ck + 1) * S_CHUNK)
        nc.sync.dma_start(out=v_sb[:, :, sl], in_=x_v[:, :, sl])

    with ExitStack() as wctx:
        w_pool = wctx.enter_context(tc.tile_pool(name="w", bufs=1))
        w_sb = [w_pool.tile([P, C], F32R, name=f"w{h}") for h in range(H)]
        wt_sb = [w_pool.tile([P, C], F32R, name=f"wt{g}") for g in range(H)]

        with ExitStack() as bctx:
            vt_pool = bctx.enter_context(tc.tile_pool(name="vt", bufs=8))
            stats = bctx.enter_context(tc.tile_pool(name="stats", bufs=1))
            psum_e = bctx.enter_context(
                tc.tile_pool(name="psum_e", bufs=1, space="PSUM"))
            psum_t = bctx.enter_context(
                tc.tile_pool(name="psum_t", bufs=6, space="PSUM"))

            # ---- Phase B: energy = v @ v.T (two PSUM banks), with the
            # transpose+copy pipelined one k ahead of the matmuls.
            e_ps = [psum_e.tile([P, C], F32, name=f"energy{h}")[:]
                    for h in range(H)]

            def make_vt(k):
                ksl = slice(k * P, (k + 1) * P)
                vt = vt_pool.tile([P, C], F32R, name="vt")
                tp = psum_t.tile([P, C], F32R, name="tp")
                for h in range(H):
                    nc.tensor.transpose(
                        tp[:, h * P:(h + 1) * P], v_sb[:, h, ksl], ident_r[:]
                    )
                if k % 2 == 0:
                    nc.scalar.copy(vt[:], tp[:])
                else:
                    nc.vector.tensor_copy(vt[:], tp[:])
                return vt

            # Two k-tiles per pipeline step: their 4 matmuls run
            # back-to-back so the self-loading weight fetch of each next
            # matmul pulls ahead of the running one (PE reorder window).
            vts = [make_vt(0), make_vt(1)]
            for k0 in range(0, KT, 2):
                for kn in (k0 + 2, k0 + 3):
                    if kn < KT:
                        vts.append(make_vt(kn))
                for k in (k0, k0 + 1):
                    vt_r = vts.pop(0)[:]
                    for h in range(H):
                        nc.tensor.matmul(
                            e_ps[h],
                            lhsT=vt_r[:, h * P:(h + 1) * P],
                            rhs=vt_r,
                            start=(k == 0),
                            stop=(k == KT - 1),
                        )

            # ---- Phase C: row softmax of (max - e) == exp(min - e)/sum.
            for h in range(H):
                mn = stats.tile([P, 1], F32, name=f"mn{h}")
                sm = stats.tile([P, 1], F32, name=f"sm{h}")
                rc = stats.tile([P, 1], F32, name=f"rc{h}")
                nc.vector.tensor_reduce(
                    mn[:], e_ps[h], axis=mybir.AxisListType.X,
                    op=mybir.AluOpType.min
                )
                nc.scalar.activation(
                    w_sb[h][:], e_ps[h], mybir.ActivationFunctionType.Exp,
                    bias=mn[:], scale=-1.0, accum_out=sm[:],
                )
                nc.vector.reciprocal(rc[:], sm[:])
                nc.vector.tensor_scalar_mul(w_sb[h][:], w_sb[h][:], rc[:])
            # wT[g][p, h*128+q] = w[h][q, g*128+p] for the second matmul.
            for g in range(H):
                for h in range(H):
                    tp2 = psum_t.tile([P, C], F32R, name="tp2", tag="tp")
                    nc.tensor.transpose(
                        tp2[:, 0:P], w_sb[h][:, g * P:(g + 1) * P], ident_r[:]
                    )
                    nc.vector.tensor_copy(
                        wt_sb[g][:, h * P:(h + 1) * P], tp2[:, 0:P])

        # ---- Phase D: y = alpha*(w @ v) + v, streamed out in 2 MB chunks.
        with ExitStack() as dctx:
            out_pool = dctx.enter_context(tc.tile_pool(name="out", bufs=3))
            psum_y = dctx.enter_context(
                tc.tile_pool(name="psum_y", bufs=4, space="PSUM"))
            for ck in range(N_CHUNKS):
                ost = out_pool.tile([P, H, S_CHUNK], F32, name="ost")
                for m in range(H):
                    for jj in range(J_PER_CHUNK):
                        j0 = ck * S_CHUNK + jj * S_TILE
                        jsl = slice(j0, j0 + S_TILE)
                        yp = psum_y.tile([P, S_TILE], F32, name="yp")
                        for g in range(H):
                            nc.tensor.matmul(
                                yp[:],
                                lhsT=wt_sb[g][:][:, m * P:(m + 1) * P],
                                rhs=v_sb[:][:, g, jsl],
                                start=(g == 0),
                                stop=(g == H - 1),
                            )
                        nc.vector.scalar_tensor_tensor(
                            out=ost[:, m, jj * S_TILE:(jj + 1) * S_TILE],
                            in0=yp[:],
                            scalar=alpha,
                            in1=v_sb[:, m, jsl].bitcast(F32),
                            op0=mybir.AluOpType.mult,
                            op1=mybir.AluOpType.add,
                        )
                if ck < N_CHUNKS - 1:
                    osl = slice(ck * S_CHUNK, (ck + 1) * S_CHUNK)
                    nc.sync.dma_start(out=y_v[:, :, osl], in_=ost[:])
                else:
                    # Split the last output chunk so the final (kernel-gating)
                    # DMA is only 512 KB.
                    q4 = S_CHUNK // 4
                    for q in range(4):
                        osl = slice(ck * S_CHUNK + q * q4,
                                    ck * S_CHUNK + (q + 1) * q4)
                        nc.sync.dma_start(out=y_v[:, :, osl],
                                          in_=ost[:, :, q * q4:(q + 1) * q4])


def _build(alpha: float) -> bass.Bass:
    # Bacc (not plain Bass): its compile() legalizes semaphore waits into
    # EventSemaphore instructions — hardware allows only 1 wait per
    # instruction and Tile freely emits more.
    nc = bacc.Bacc("TRN2", target_bir_lowering=False)
    # x is declared float32r (same 32-bit layout as fp32 at rest) so the DMA,
    # the PE transposes, and both matmuls form a consistent f32r chain for
    # the BIR verifier; the PE truncates to FP22 on read either way.
    x = nc.dram_tensor("x", [C, HW], F32R, kind="ExternalInput")
    y = nc.dram_tensor("y", [C, HW], F32, kind="ExternalOutput")
    x_v = x.rearrange("(h p) s -> p h s", p=P)
    y_v = y.rearrange("(h p) s -> p h s", p=P)

    with tile.TileContext(nc) as tc, ExitStack() as ctx:
        singles = ctx.enter_context(tc.tile_pool(name="singles", bufs=1))
        ident = singles.tile([P, P], F32, name="ident")
        make_identity(nc, ident)
        ident_r = singles.tile([P, P], F32R, name="ident_r")
        nc.vector.tensor_copy(ident_r[:], ident[:])
        # Whole v resident: 128 KB per partition.
        v_sb = singles.tile([P, H, HW], F32R, name="v_sb")
        emit(nc, tc, alpha, ident_r, v_sb, x_v, y_v)
    nc.compile()
    return nc


def kernel(x: np.ndarray, alpha: np.ndarray, **_kw) -> np.ndarray:
    assert x.shape == (B, C, HH, WW) and x.dtype == np.float32
    xs = np.ascontiguousarray(x.reshape(B, C, HW)).astype(np.float32, copy=False)
    nc = _build(float(np.asarray(alpha).reshape(-1)[0]))
    in_maps = [{"x": xs[b]} for b in range(B)]
    res = run_bass_kernel_spmd(nc, in_maps, core_ids=list(range(B)))
    out = np.stack([np.asarray(r["y"]) for r in res.results])
    return out.reshape(B, C, HH, WW).astype(np.float32, copy=False)



# revision 13
# speedup vs baseline: 1.3490x; 1.3490x over previous
"""Trainium2 Bass kernel for a channel-attention block.

Per batch b (one NeuronCore each, 8 total):
    v      = x[b].reshape(C, H*W)                    # [256, 16384]
    energy = v @ v.T                                 # [256, 256]
    w      = softmax(max(energy, -1) - energy, -1)   # == softmax(-energy)
    y      = alpha * (w @ v) + x[b]

Layout / strategy (per core):
  - v stays resident in SBUF as [128, 2, 16384] f32r (c = h*128 + p).
  - energy needs s on partitions, so each 128-wide s-tile of v is
    transposed on the PE, then fed to two float32r matmuls (FP22
    truncation, full bf16-rate at N>=256) accumulating [128, 256] PSUM
    tiles over all 128 s-tiles.  The transpose+copy for k+1 is emitted
    one iteration ahead of k's matmuls (software pipeline) and the
    PSUM->SBUF copies alternate ScalarE/VectorE.
  - Stable softmax via one reduce-min + one fused ScalarE
    exp(-energy + rowmin) with accumulated row-sum, then reciprocal
    multiply.  (softmax(max-e) == exp(rowmin-e)/sum.)
  - The residual is folded into the weights: W' = alpha*softmax + I, so
    the second matmul computes y = W' @ v directly and phase D needs only
    PSUM->SBUF copies (alternating ScalarE/VectorE) before the output
    DMAs.  D runs m-major so its first chunk depends only on h=0's
    softmax; first/last chunks stream per 512 KB j-tile, the rest per
    1 MB.  PSUM pools are scoped per phase: B/C get 6 transpose banks,
    D gets 4 output banks.
"""

from contextlib import ExitStack

import numpy as np

import concourse.bass as bass
import concourse.mybir as mybir
import concourse.tile as tile
from concourse import bacc
from concourse.bass_utils import run_bass_kernel_spmd
from concourse.masks import make_identity

B, C, HH, WW = 8, 256, 128, 128
HW = HH * WW            # 16384
P = 128
H = C // P              # 2 channel chunks
KT = HW // P            # 128 contraction tiles for energy
S_CHUNK = 2048          # columns per streaming DMA chunk (2 MB)
N_CHUNKS = HW // S_CHUNK
S_TILE = 512            # second-matmul moving free dim (1 PSUM bank)
J_PER_CHUNK = S_CHUNK // S_TILE

F32 = mybir.dt.float32
F32R = mybir.dt.float32r


def emit(nc, tc, alpha, ident_r, v_sb, x_v, y_v, phases="ABCD"):
    """One full per-core pass (phases A-D). Pools are scoped inside.

    phases: subset of "ABCD" — used only by the phase-bisect timing
    harness; correctness requires all four.
    """
    do_a = "A" in phases
    do_b = "B" in phases
    do_c = "C" in phases
    do_d = "D" in phases
    # ---- Phase A: stream x into SBUF.  The PE outruns the DMA early on
    # (8.7 us/chunk of transpose+matmul vs 5.7 us/chunk of DMA), so the
    # first chunk is quartered and the second halved to cut PE starvation.
    if do_a:
        for q in range(4):
            sl = slice(q * (S_CHUNK // 4), (q + 1) * (S_CHUNK // 4))
            nc.sync.dma_start(out=v_sb[:, :, sl], in_=x_v[:, :, sl])
        for hh in range(2):
            sl = slice(S_CHUNK + hh * (S_CHUNK // 2),
                       S_CHUNK + (hh + 1) * (S_CHUNK // 2))
            nc.sync.dma_start(out=v_sb[:, :, sl], in_=x_v[:, :, sl])
        for ck in range(2, N_CHUNKS):
            sl = slice(ck * S_CHUNK, (ck + 1) * S_CHUNK)
            nc.sync.dma_start(out=v_sb[:, :, sl], in_=x_v[:, :, sl])

    with ExitStack() as wctx:
        w_pool = wctx.enter_context(tc.tile_pool(name="w", bufs=1))
        w_sb = [w_pool.tile([P, C], F32R, name=f"w{h}") for h in range(H)]
        wt_sb = [w_pool.tile([P, C], F32R, name=f"wt{g}") for g in range(H)]

        with ExitStack() as bctx:
            vt_pool = bctx.enter_context(tc.tile_pool(name="vt", bufs=8))
            stats = bctx.enter_context(tc.tile_pool(name="stats", bufs=1))
            psum_e = bctx.enter_context(
                tc.tile_pool(name="psum_e", bufs=1, space="PSUM"))
            psum_t = bctx.enter_context(
                tc.tile_pool(name="psum_t", bufs=6, space="PSUM"))

            # ---- Phase B: energy = v @ v.T (two PSUM banks), with the
            # transpose+copy pipelined one k ahead of the matmuls.
            e_ps = [psum_e.tile([P, C], F32, name=f"energy{h}")[:]
                    for h in range(H)]

            def make_vt(k):
                ksl = slice(k * P, (k + 1) * P)
                vt = vt_pool.tile([P, C], F32R, name="vt")
                tp = psum_t.tile([P, C], F32R, name="tp")
                for h in range(H):
                    nc.tensor.transpose(
                        tp[:, h * P:(h + 1) * P], v_sb[:, h, ksl], ident_r[:]
                    )
                if k % 2 == 0:
                    nc.scalar.copy(vt[:], tp[:])
                else:
                    nc.vector.tensor_copy(vt[:], tp[:])
                return vt

            # Two k-tiles per pipeline step: their 4 matmuls run
            # back-to-back so the self-loading weight fetch of each next
            # matmul pulls ahead of the running one (PE reorder window).
            if do_b:
                vts = [make_vt(0), make_vt(1)]
                for k0 in range(0, KT, 2):
                    for kn in (k0 + 2, k0 + 3):
                        if kn < KT:
                            vts.append(make_vt(kn))
                    for k in (k0, k0 + 1):
                        vt_r = vts.pop(0)[:]
                        for h in range(H):
                            nc.tensor.matmul(
                                e_ps[h],
                                lhsT=vt_r[:, h * P:(h + 1) * P],
                                rhs=vt_r,
                                start=(k == 0),
                                stop=(k == KT - 1),
                            )

            # ---- Phase C: row softmax of (max - e) == exp(min - e)/sum,
            # then W' = alpha*softmax + I so phase D's matmul computes
            # alpha*(w@v) + v directly and needs no elementwise pass.
            # Each h's wT transposes follow immediately so everything phase
            # D's m=0 needs is ahead of h=1 work in engine order.
            # wT[g][p, h*128+q] = W'[h][q, g*128+p] for the second matmul.
            for h in range(H if do_c else 0):
                mn = stats.tile([P, 1], F32, name=f"mn{h}")
                sm = stats.tile([P, 1], F32, name=f"sm{h}")
                rc = stats.tile([P, 1], F32, name=f"rc{h}")
                nc.vector.tensor_reduce(
                    mn[:], e_ps[h], axis=mybir.AxisListType.X,
                    op=mybir.AluOpType.min
                )
                nc.scalar.activation(
                    w_sb[h][:], e_ps[h], mybir.ActivationFunctionType.Exp,
                    bias=mn[:], scale=-1.0, accum_out=sm[:],
                )
                nc.vector.reciprocal(rc[:], sm[:])
                nc.vector.tensor_scalar_mul(rc[:], rc[:], alpha)
                nc.vector.tensor_scalar_mul(w_sb[h][:], w_sb[h][:], rc[:])
                nc.vector.tensor_tensor(
                    out=w_sb[h][:, h * P:(h + 1) * P],
                    in0=w_sb[h][:, h * P:(h + 1) * P],
                    in1=ident_r[:],
                    op=mybir.AluOpType.add,
                )
                for g in range(H):
                    tp2 = psum_t.tile([P, C], F32R, name="tp2", tag="tp")
                    nc.tensor.transpose(
                        tp2[:, 0:P], w_sb[h][:, g * P:(g + 1) * P], ident_r[:]
                    )
                    if g == 0:
                        nc.vector.tensor_copy(
                            wt_sb[g][:, h * P:(h + 1) * P], tp2[:, 0:P])
                    else:
                        nc.scalar.copy(
                            wt_sb[g][:, h * P:(h + 1) * P], tp2[:, 0:P])

        # ---- Phase D: y = W' @ v (residual and alpha folded into W'),
        # m-major so the first output chunk only needs h=0's
        # softmax+transposes, streamed out per (m, ck) in 1 MB chunks.
        # PSUM->SBUF copies alternate ScalarE/VectorE (GPSIMD cannot read
        # PSUM); one engine alone would pace the output DMA stream.
        with ExitStack() as dctx:
            out_pool = dctx.enter_context(tc.tile_pool(name="out", bufs=4))
            psum_y = dctx.enter_context(
                tc.tile_pool(name="psum_y", bufs=4, space="PSUM"))
            n_cp = 0
            for m in range(H if do_d else 0):
                for ck in range(N_CHUNKS):
                    ost = out_pool.tile([P, S_CHUNK], F32, name="ost")
                    for jj in range(J_PER_CHUNK):
                        j0 = ck * S_CHUNK + jj * S_TILE
                        jsl = slice(j0, j0 + S_TILE)
                        yp = psum_y.tile([P, S_TILE], F32, name="yp")
                        for g in range(H):
                            nc.tensor.matmul(
                                yp[:],
                                lhsT=wt_sb[g][:][:, m * P:(m + 1) * P],
                                rhs=v_sb[:][:, g, jsl],
                                start=(g == 0),
                                stop=(g == H - 1),
                            )
                        osb = ost[:, jj * S_TILE:(jj + 1) * S_TILE]
                        if n_cp % 2 == 0:
                            nc.scalar.copy(osb, yp[:])
                        else:
                            nc.vector.tensor_copy(osb, yp[:])
                        n_cp += 1
                        # First and last (m, ck): fire each 512 KB j-tile as
                        # its own DMA so the output stream starts as early
                        # and drains as late-gating-free as possible.
                        if (m, ck) in ((0, 0), (H - 1, N_CHUNKS - 1)):
                            osl = slice(j0, j0 + S_TILE)
                            nc.sync.dma_start(
                                out=y_v[:, m, osl],
                                in_=ost[:, jj * S_TILE:(jj + 1) * S_TILE])
                    if (m, ck) not in ((0, 0), (H - 1, N_CHUNKS - 1)):
                        osl = slice(ck * S_CHUNK, (ck + 1) * S_CHUNK)
                        nc.sync.dma_start(out=y_v[:, m, osl], in_=ost[:])


def _build(alpha: float) -> bass.Bass:
    # Bacc (not plain Bass): its compile() legalizes semaphore waits into
    # EventSemaphore instructions — hardware allows only 1 wait per
    # instruction and Tile freely emits more.
    nc = bacc.Bacc("TRN2", target_bir_lowering=False)
    # x is declared float32r (same 32-bit layout as fp32 at rest) so the DMA,
    # the PE transposes, and both matmuls form a consistent f32r chain for
    # the BIR verifier; the PE truncates to FP22 on read either way.
    x = nc.dram_tensor("x", [C, HW], F32R, kind="ExternalInput")
    y = nc.dram_tensor("y", [C, HW], F32, kind="ExternalOutput")
    x_v = x.rearrange("(h p) s -> p h s", p=P)
    y_v = y.rearrange("(h p) s -> p h s", p=P)

    with tile.TileContext(nc) as tc, ExitStack() as ctx:
        singles = ctx.enter_context(tc.tile_pool(name="singles", bufs=1))
        ident = singles.tile([P, P], F32, name="ident")
        make_identity(nc, ident)
        ident_r = singles.tile([P, P], F32R, name="ident_r")
        nc.vector.tensor_copy(ident_r[:], ident[:])
        # Whole v resident: 128 KB per partition.
        v_sb = singles.tile([P, H, HW], F32R, name="v_sb")
        emit(nc, tc, alpha, ident_r, v_sb, x_v, y_v)
    nc.compile()
    return nc


def kernel(x: np.ndarray, alpha: np.ndarray, **_kw) -> np.ndarray:
    assert x.shape == (B, C, HH, WW) and x.dtype == np.float32
    xs = np.ascontiguousarray(x.reshape(B, C, HW)).astype(np.float32, copy=False)
    nc = _build(float(np.asarray(alpha).reshape(-1)[0]))
    in_maps = [{"x": xs[b]} for b in range(B)]
    res = run_bass_kernel_spmd(nc, in_maps, core_ids=list(range(B)))
    out = np.stack([np.asarray(r["y"]) for r in res.results])
    return out.reshape(B, C, HH, WW).astype(np.float32, copy=False)



# revision 14
# speedup vs baseline: 1.7020x; 1.2617x over previous
"""Trainium2 Bass kernel for a channel-attention block.

Per batch b (one NeuronCore each, 8 total):
    v      = x[b].reshape(C, H*W)                    # [256, 16384]
    energy = v @ v.T                                 # [256, 256]
    w      = softmax(max(energy, -1) - energy, -1)   # == softmax(-energy)
    y      = alpha * (w @ v) + x[b]

Layout / strategy (per core):
  - v stays resident in SBUF as [128, 2, 16384] f32r (c = h*128 + p).
  - energy needs s on partitions, so each 128-wide s-tile of v is
    transposed on the PE, then fed to two float32r matmuls (FP22
    truncation, full bf16-rate at N>=256) accumulating [128, 256] PSUM
    tiles over all 128 s-tiles.  The transpose+copy for k+1 is emitted
    one iteration ahead of k's matmuls (software pipeline) and the
    PSUM->SBUF copies alternate ScalarE/VectorE.
  - Stable softmax via one reduce-min + one fused ScalarE
    exp(-energy + rowmin) with accumulated row-sum, then reciprocal
    multiply.  (softmax(max-e) == exp(rowmin-e)/sum.)
  - The residual is folded into the weights: W' = alpha*softmax + I, so
    the second matmul computes y = W' @ v directly and phase D needs only
    PSUM->SBUF copies (alternating ScalarE/VectorE) before the output
    DMAs.  D runs m-major so its first chunk depends only on h=0's
    softmax; first/last chunks stream per 512 KB j-tile, the rest per
    1 MB.  PSUM pools are scoped per phase: B/C get 6 transpose banks,
    D gets 4 output banks.
"""

from contextlib import ExitStack

import numpy as np

import concourse.bass as bass
import concourse.mybir as mybir
import concourse.tile as tile
from concourse import bacc
from concourse.bass_utils import run_bass_kernel_spmd
from concourse.masks import make_identity

B, C, HH, WW = 8, 256, 128, 128
HW = HH * WW            # 16384
P = 128
H = C // P              # 2 channel chunks
KT = HW // P            # 128 contraction tiles for energy
S_CHUNK = 2048          # columns per streaming DMA chunk (2 MB)
N_CHUNKS = HW // S_CHUNK
S_TILE = 512            # second-matmul moving free dim (1 PSUM bank)
J_PER_CHUNK = S_CHUNK // S_TILE

F32 = mybir.dt.float32
F32R = mybir.dt.float32r


def emit(nc, tc, alpha, ident_r, v_sb, x_v, y_v, phases="ABCD"):
    """One full per-core pass (phases A-D). Pools are scoped inside.

    phases: subset of "ABCD" — used only by the phase-bisect timing
    harness; correctness requires all four.
    """
    do_a = "A" in phases
    do_b = "B" in phases
    do_c = "C" in phases
    do_d = "D" in phases
    # ---- Phase A: stream x into SBUF.  The PE outruns the DMA early on
    # (8.7 us/chunk of transpose+matmul vs 5.7 us/chunk of DMA), so the
    # first chunk is quartered and the second halved to cut PE starvation.
    if do_a:
        for q in range(4):
            sl = slice(q * (S_CHUNK // 4), (q + 1) * (S_CHUNK // 4))
            nc.sync.dma_start(out=v_sb[:, :, sl], in_=x_v[:, :, sl])
        for hh in range(2):
            sl = slice(S_CHUNK + hh * (S_CHUNK // 2),
                       S_CHUNK + (hh + 1) * (S_CHUNK // 2))
            nc.sync.dma_start(out=v_sb[:, :, sl], in_=x_v[:, :, sl])
        for ck in range(2, N_CHUNKS):
            sl = slice(ck * S_CHUNK, (ck + 1) * S_CHUNK)
            nc.sync.dma_start(out=v_sb[:, :, sl], in_=x_v[:, :, sl])

    with ExitStack() as wctx:
        w_pool = wctx.enter_context(tc.tile_pool(name="w", bufs=1))
        w_sb = [w_pool.tile([P, C], F32R, name=f"w{h}") for h in range(H)]
        wt_sb = [w_pool.tile([P, C], F32R, name=f"wt{g}") for g in range(H)]

        with ExitStack() as bctx:
            vt_pool = bctx.enter_context(tc.tile_pool(name="vt", bufs=8))
            stats = bctx.enter_context(tc.tile_pool(name="stats", bufs=1))
            psum_e = bctx.enter_context(
                tc.tile_pool(name="psum_e", bufs=1, space="PSUM"))
            psum_t = bctx.enter_context(
                tc.tile_pool(name="psum_t", bufs=6, space="PSUM"))

            # ---- Phase B: energy = v @ v.T (two PSUM banks), with the
            # transpose+copy pipelined one k ahead of the matmuls.
            e_ps = [psum_e.tile([P, C], F32, name=f"energy{h}")[:]
                    for h in range(H)]

            def make_vt(k):
                ksl = slice(k * P, (k + 1) * P)
                vt = vt_pool.tile([P, C], F32R, name="vt")
                tp = psum_t.tile([P, C], F32R, name="tp")
                for h in range(H):
                    nc.tensor.transpose(
                        tp[:, h * P:(h + 1) * P], v_sb[:, h, ksl], ident_r[:]
                    )
                if k % 2 == 0:
                    nc.scalar.copy(vt[:], tp[:])
                else:
                    nc.vector.tensor_copy(vt[:], tp[:])
                return vt

            # Two k-tiles per pipeline step: their 4 matmuls run
            # back-to-back so the self-loading weight fetch of each next
            # matmul pulls ahead of the running one (PE reorder window).
            if do_b:
                vts = [make_vt(0), make_vt(1)]
                for k0 in range(0, KT, 2):
                    for kn in (k0 + 2, k0 + 3):
                        if kn < KT:
                            vts.append(make_vt(kn))
                    for k in (k0, k0 + 1):
                        vt_r = vts.pop(0)[:]
                        for h in range(H):
                            nc.tensor.matmul(
                                e_ps[h],
                                lhsT=vt_r[:, h * P:(h + 1) * P],
                                rhs=vt_r,
                                start=(k == 0),
                                stop=(k == KT - 1),
                            )

            # ---- Phase C: row softmax of (max - e) == exp(min - e)/sum,
            # then W' = alpha*softmax + I so phase D's matmul computes
            # alpha*(w@v) + v directly and needs no elementwise pass.
            # Each h's wT transposes follow immediately so everything phase
            # D's m=0 needs is ahead of h=1 work in engine order.
            # wT[g][p, h*128+q] = W'[h][q, g*128+p] for the second matmul.
            for h in range(H if do_c else 0):
                mn = stats.tile([P, 1], F32, name=f"mn{h}")
                sm = stats.tile([P, 1], F32, name=f"sm{h}")
                rc = stats.tile([P, 1], F32, name=f"rc{h}")
                nc.vector.tensor_reduce(
                    mn[:], e_ps[h], axis=mybir.AxisListType.X,
                    op=mybir.AluOpType.min
                )
                nc.scalar.activation(
                    w_sb[h][:], e_ps[h], mybir.ActivationFunctionType.Exp,
                    bias=mn[:], scale=-1.0, accum_out=sm[:],
                )
                nc.vector.reciprocal(rc[:], sm[:])
                nc.vector.tensor_scalar_mul(rc[:], rc[:], alpha)
                nc.vector.tensor_scalar_mul(w_sb[h][:], w_sb[h][:], rc[:])
                nc.vector.tensor_tensor(
                    out=w_sb[h][:, h * P:(h + 1) * P],
                    in0=w_sb[h][:, h * P:(h + 1) * P],
                    in1=ident_r[:],
                    op=mybir.AluOpType.add,
                )
                for g in range(H):
                    tp2 = psum_t.tile([P, C], F32R, name="tp2", tag="tp")
                    nc.tensor.transpose(
                        tp2[:, 0:P], w_sb[h][:, g * P:(g + 1) * P], ident_r[:]
                    )
                    if g == 0:
                        nc.vector.tensor_copy(
                            wt_sb[g][:, h * P:(h + 1) * P], tp2[:, 0:P])
                    else:
                        nc.scalar.copy(
                            wt_sb[g][:, h * P:(h + 1) * P], tp2[:, 0:P])

        # ---- Phase D: y = W' @ v (residual and alpha folded into W'),
        # m-major so the first output tiles only need h=0's
        # softmax+transposes.  Each 256 KB j-tile is copied PSUM->SBUF
        # (VectorE:ScalarE 2:1 — GPSIMD cannot read PSUM, and one engine
        # alone would pace the output stream) and fired as its own DMA so
        # the output stream starts early and nothing gates on 1 MB chunks.
        with ExitStack() as dctx:
            out_pool = dctx.enter_context(tc.tile_pool(name="out", bufs=8))
            psum_y = dctx.enter_context(
                tc.tile_pool(name="psum_y", bufs=6, space="PSUM"))
            n_cp = 0
            for m in range(H if do_d else 0):
                for jt in range(HW // S_TILE):
                    j0 = jt * S_TILE
                    jsl = slice(j0, j0 + S_TILE)
                    yp = psum_y.tile([P, S_TILE], F32, name="yp")
                    for g in range(H):
                        nc.tensor.matmul(
                            yp[:],
                            lhsT=wt_sb[g][:][:, m * P:(m + 1) * P],
                            rhs=v_sb[:][:, g, jsl],
                            start=(g == 0),
                            stop=(g == H - 1),
                        )
                    ost = out_pool.tile([P, S_TILE], F32, name="ost")
                    if n_cp % 3 == 2:
                        nc.scalar.copy(ost[:], yp[:])
                    else:
                        nc.vector.tensor_copy(ost[:], yp[:])
                    n_cp += 1
                    nc.sync.dma_start(out=y_v[:, m, jsl], in_=ost[:])


def _build(alpha: float) -> bass.Bass:
    # Bacc (not plain Bass): its compile() legalizes semaphore waits into
    # EventSemaphore instructions — hardware allows only 1 wait per
    # instruction and Tile freely emits more.
    nc = bacc.Bacc("TRN2", target_bir_lowering=False)
    # x is declared float32r (same 32-bit layout as fp32 at rest) so the DMA,
    # the PE transposes, and both matmuls form a consistent f32r chain for
    # the BIR verifier; the PE truncates to FP22 on read either way.
    x = nc.dram_tensor("x", [C, HW], F32R, kind="ExternalInput")
    y = nc.dram_tensor("y", [C, HW], F32, kind="ExternalOutput")
    x_v = x.rearrange("(h p) s -> p h s", p=P)
    y_v = y.rearrange("(h p) s -> p h s", p=P)

    with tile.TileContext(nc) as tc, ExitStack() as ctx:
        singles = ctx.enter_context(tc.tile_pool(name="singles", bufs=1))
        ident = singles.tile([P, P], F32, name="ident")
        make_identity(nc, ident)
        ident_r = singles.tile([P, P], F32R, name="ident_r")
        nc.vector.tensor_copy(ident_r[:], ident[:])
        # Whole v resident: 128 KB per partition.
        v_sb = singles.tile([P, H, HW], F32R, name="v_sb")
        emit(nc, tc, alpha, ident_r, v_sb, x_v, y_v)
    nc.compile()
    return nc


def kernel(x: np.ndarray, alpha: np.ndarray, **_kw) -> np.ndarray:
    assert x.shape == (B, C, HH, WW) and x.dtype == np.float32
    xs = np.ascontiguousarray(x.reshape(B, C, HW)).astype(np.float32, copy=False)
    nc = _build(float(np.asarray(alpha).reshape(-1)[0]))
    in_maps = [{"x": xs[b]} for b in range(B)]
    res = run_bass_kernel_spmd(nc, in_maps, core_ids=list(range(B)))
    out = np.stack([np.asarray(r["y"]) for r in res.results])
    return out.reshape(B, C, HH, WW).astype(np.float32, copy=False)



# revision 15
# speedup vs baseline: 2.1606x; 1.2694x over previous
"""Trainium2 Bass kernel for a channel-attention block.

Per batch b (one NeuronCore each, 8 total):
    v      = x[b].reshape(C, H*W)                    # [256, 16384]
    energy = v @ v.T                                 # [256, 256]
    w      = softmax(max(energy, -1) - energy, -1)   # == softmax(-energy)
    y      = alpha * (w @ v) + x[b]

Layout / strategy (per core):
  - v stays resident in SBUF as [128, 2, 16384] f32r (c = h*128 + p).
  - energy needs s on partitions, so each 128-wide s-tile of v is
    transposed on the PE, then fed to two float32r matmuls (FP22
    truncation, full bf16-rate at N>=256) accumulating [128, 256] PSUM
    tiles over all 128 s-tiles.  The transpose+copy for k+1 is emitted
    one iteration ahead of k's matmuls (software pipeline) and the
    PSUM->SBUF copies alternate ScalarE/VectorE.
  - Stable softmax via one reduce-min + one fused ScalarE
    exp(-energy + rowmin) with accumulated row-sum, then reciprocal
    multiply.  (softmax(max-e) == exp(rowmin-e)/sum.)
  - The residual is folded into the weights: W' = alpha*softmax + I, so
    the second matmul computes y = W' @ v directly and phase D needs only
    PSUM->SBUF copies (alternating ScalarE/VectorE) before the output
    DMAs.  D runs m-major so its first chunk depends only on h=0's
    softmax; first/last chunks stream per 512 KB j-tile, the rest per
    1 MB.  PSUM pools are scoped per phase: B/C get 6 transpose banks,
    D gets 4 output banks.
"""

from contextlib import ExitStack

import numpy as np

import concourse.bass as bass
import concourse.mybir as mybir
import concourse.tile as tile
from concourse import bacc
from concourse.bass_utils import run_bass_kernel_spmd
from concourse.masks import make_identity

B, C, HH, WW = 8, 256, 128, 128
HW = HH * WW            # 16384
P = 128
H = C // P              # 2 channel chunks
KT = HW // P            # 128 contraction tiles for energy
S_CHUNK = 2048          # columns per streaming DMA chunk (2 MB)
N_CHUNKS = HW // S_CHUNK
S_TILE = 512            # second-matmul moving free dim (1 PSUM bank)
J_PER_CHUNK = S_CHUNK // S_TILE

F32 = mybir.dt.float32
F32R = mybir.dt.float32r


def emit(nc, tc, alpha, ident_r, v_sb, x_v, y_v, phases="ABCD"):
    """One full per-core pass (phases A-D). Pools are scoped inside.

    phases: subset of "ABCD" — used only by the phase-bisect timing
    harness; correctness requires all four.
    """
    do_a = "A" in phases
    do_b = "B" in phases
    do_c = "C" in phases
    do_d = "D" in phases
    # ---- Phase A: stream x into SBUF.  The PE outruns the DMA early on
    # (8.7 us/chunk of transpose+matmul vs 5.7 us/chunk of DMA), so the
    # first chunk is quartered and the second halved to cut PE starvation.
    if do_a:
        for q in range(4):
            sl = slice(q * (S_CHUNK // 4), (q + 1) * (S_CHUNK // 4))
            nc.sync.dma_start(out=v_sb[:, :, sl], in_=x_v[:, :, sl])
        for hh in range(2):
            sl = slice(S_CHUNK + hh * (S_CHUNK // 2),
                       S_CHUNK + (hh + 1) * (S_CHUNK // 2))
            nc.sync.dma_start(out=v_sb[:, :, sl], in_=x_v[:, :, sl])
        for ck in range(2, N_CHUNKS):
            sl = slice(ck * S_CHUNK, (ck + 1) * S_CHUNK)
            nc.sync.dma_start(out=v_sb[:, :, sl], in_=x_v[:, :, sl])

    with ExitStack() as wctx:
        w_pool = wctx.enter_context(tc.tile_pool(name="w", bufs=1))
        w_sb = [w_pool.tile([P, C], F32R, name=f"w{h}") for h in range(H)]
        wt_sb = [w_pool.tile([P, C], F32R, name=f"wt{g}") for g in range(H)]

        with ExitStack() as bctx:
            vt_pool = bctx.enter_context(tc.tile_pool(name="vt", bufs=8))
            stats = bctx.enter_context(tc.tile_pool(name="stats", bufs=1))
            psum_e = bctx.enter_context(
                tc.tile_pool(name="psum_e", bufs=1, space="PSUM"))
            psum_t = bctx.enter_context(
                tc.tile_pool(name="psum_t", bufs=6, space="PSUM"))

            # ---- Phase B: energy = v @ v.T (two PSUM banks), with the
            # transpose+copy pipelined one k ahead of the matmuls.
            e_ps = [psum_e.tile([P, C], F32, name=f"energy{h}")[:]
                    for h in range(H)]

            def make_vt(k):
                ksl = slice(k * P, (k + 1) * P)
                vt = vt_pool.tile([P, C], F32R, name="vt")
                tp = psum_t.tile([P, C], F32R, name="tp")
                for h in range(H):
                    nc.tensor.transpose(
                        tp[:, h * P:(h + 1) * P], v_sb[:, h, ksl], ident_r[:]
                    )
                if k % 2 == 0:
                    nc.scalar.copy(vt[:], tp[:])
                else:
                    nc.vector.tensor_copy(vt[:], tp[:])
                return vt

            # Two k-tiles per pipeline step: their 4 matmuls run
            # back-to-back so the self-loading weight fetch of each next
            # matmul pulls ahead of the running one (PE reorder window).
            if do_b:
                vts = [make_vt(0), make_vt(1)]
                for k0 in range(0, KT, 2):
                    for kn in (k0 + 2, k0 + 3):
                        if kn < KT:
                            vts.append(make_vt(kn))
                    for k in (k0, k0 + 1):
                        vt_r = vts.pop(0)[:]
                        for h in range(H):
                            nc.tensor.matmul(
                                e_ps[h],
                                lhsT=vt_r[:, h * P:(h + 1) * P],
                                rhs=vt_r,
                                start=(k == 0),
                                stop=(k == KT - 1),
                            )

            # ---- Phase C: row softmax of (max - e) == exp(min - e)/sum,
            # then W' = alpha*softmax + I so phase D's matmul computes
            # alpha*(w@v) + v directly and needs no elementwise pass.
            # Each h's wT transposes follow immediately so everything phase
            # D's m=0 needs is ahead of h=1 work in engine order.
            # wT[g][p, h*128+q] = W'[h][q, g*128+p] for the second matmul.
            for h in range(H if do_c else 0):
                mn = stats.tile([P, 1], F32, name=f"mn{h}")
                sm = stats.tile([P, 1], F32, name=f"sm{h}")
                rc = stats.tile([P, 1], F32, name=f"rc{h}")
                nc.vector.tensor_reduce(
                    mn[:], e_ps[h], axis=mybir.AxisListType.X,
                    op=mybir.AluOpType.min
                )
                nc.scalar.activation(
                    w_sb[h][:], e_ps[h], mybir.ActivationFunctionType.Exp,
                    bias=mn[:], scale=-1.0, accum_out=sm[:],
                )
                nc.vector.reciprocal(rc[:], sm[:])
                nc.vector.tensor_scalar_mul(rc[:], rc[:], alpha)
                nc.vector.tensor_scalar_mul(w_sb[h][:], w_sb[h][:], rc[:])
                nc.vector.tensor_tensor(
                    out=w_sb[h][:, h * P:(h + 1) * P],
                    in0=w_sb[h][:, h * P:(h + 1) * P],
                    in1=ident_r[:],
                    op=mybir.AluOpType.add,
                )
                for g in range(H):
                    tp2 = psum_t.tile([P, C], F32R, name="tp2", tag="tp")
                    nc.tensor.transpose(
                        tp2[:, 0:P], w_sb[h][:, g * P:(g + 1) * P], ident_r[:]
                    )
                    if g == 0:
                        nc.vector.tensor_copy(
                            wt_sb[g][:, h * P:(h + 1) * P], tp2[:, 0:P])
                    else:
                        nc.scalar.copy(
                            wt_sb[g][:, h * P:(h + 1) * P], tp2[:, 0:P])

        # ---- Phase D: y = W' @ v (residual and alpha folded into W').
        # jt-major: both m-blocks of a j-range are computed back to back so
        # v_sb columns are released left to right — the next loop
        # iteration's input DMA (which overlaps this iteration's tail; the
        # For_i barrier syncs engines, not in-flight DMA rings) can refill
        # v_sb behind us.  Each 256 KB j-tile is copied PSUM->SBUF
        # (VectorE:ScalarE 2:1 — GPSIMD cannot read PSUM, and one engine
        # alone would pace the output stream) and fired as its own DMA.
        with ExitStack() as dctx:
            out_pool = dctx.enter_context(tc.tile_pool(name="out", bufs=8))
            psum_y = dctx.enter_context(
                tc.tile_pool(name="psum_y", bufs=6, space="PSUM"))
            n_cp = 0
            for jt in range(HW // S_TILE if do_d else 0):
                j0 = jt * S_TILE
                jsl = slice(j0, j0 + S_TILE)
                for m in range(H):
                    yp = psum_y.tile([P, S_TILE], F32, name="yp")
                    for g in range(H):
                        nc.tensor.matmul(
                            yp[:],
                            lhsT=wt_sb[g][:][:, m * P:(m + 1) * P],
                            rhs=v_sb[:][:, g, jsl],
                            start=(g == 0),
                            stop=(g == H - 1),
                        )
                    ost = out_pool.tile([P, S_TILE], F32, name="ost")
                    if n_cp % 3 == 2:
                        nc.scalar.copy(ost[:], yp[:])
                    else:
                        nc.vector.tensor_copy(ost[:], yp[:])
                    n_cp += 1
                    nc.sync.dma_start(out=y_v[:, m, jsl], in_=ost[:])


def _build(alpha: float) -> bass.Bass:
    # Bacc (not plain Bass): its compile() legalizes semaphore waits into
    # EventSemaphore instructions — hardware allows only 1 wait per
    # instruction and Tile freely emits more.
    nc = bacc.Bacc("TRN2", target_bir_lowering=False)
    # x is declared float32r (same 32-bit layout as fp32 at rest) so the DMA,
    # the PE transposes, and both matmuls form a consistent f32r chain for
    # the BIR verifier; the PE truncates to FP22 on read either way.
    x = nc.dram_tensor("x", [C, HW], F32R, kind="ExternalInput")
    y = nc.dram_tensor("y", [C, HW], F32, kind="ExternalOutput")
    x_v = x.rearrange("(h p) s -> p h s", p=P)
    y_v = y.rearrange("(h p) s -> p h s", p=P)

    with tile.TileContext(nc) as tc, ExitStack() as ctx:
        singles = ctx.enter_context(tc.tile_pool(name="singles", bufs=1))
        ident = singles.tile([P, P], F32, name="ident")
        make_identity(nc, ident)
        ident_r = singles.tile([P, P], F32R, name="ident_r")
        nc.vector.tensor_copy(ident_r[:], ident[:])
        # Whole v resident: 128 KB per partition.
        v_sb = singles.tile([P, H, HW], F32R, name="v_sb")
        emit(nc, tc, alpha, ident_r, v_sb, x_v, y_v)
    nc.compile()
    return nc


def kernel(x: np.ndarray, alpha: np.ndarray, **_kw) -> np.ndarray:
    assert x.shape == (B, C, HH, WW) and x.dtype == np.float32
    xs = np.ascontiguousarray(x.reshape(B, C, HW)).astype(np.float32, copy=False)
    nc = _build(float(np.asarray(alpha).reshape(-1)[0]))
    in_maps = [{"x": xs[b]} for b in range(B)]
    res = run_bass_kernel_spmd(nc, in_maps, core_ids=list(range(B)))
    out = np.stack([np.asarray(r["y"]) for r in res.results])
    return out.reshape(B, C, HH, WW).astype(np.float32, copy=False)



# revision 17
# speedup vs baseline: 2.2853x; 1.0577x over previous
"""Trainium2 Bass kernel for a channel-attention block.

Per batch b (one NeuronCore each, 8 total):
    v      = x[b].reshape(C, H*W)                    # [256, 16384]
    energy = v @ v.T                                 # [256, 256]
    w      = softmax(max(energy, -1) - energy, -1)   # == softmax(-energy)
    y      = alpha * (w @ v) + x[b]

Layout / strategy (per core):
  - v stays resident in SBUF as [128, 2, 16384] f32r (c = h*128 + p).
  - energy needs s on partitions, so each 128-wide s-tile of v is
    transposed on the PE, then fed to two float32r matmuls (FP22
    truncation, full bf16-rate at N>=256) accumulating [128, 256] PSUM
    tiles over all 128 s-tiles.  The transpose+copy for k+1 is emitted
    one iteration ahead of k's matmuls (software pipeline) and the
    PSUM->SBUF copies alternate ScalarE/VectorE.
  - Stable softmax via one reduce-min + one fused ScalarE
    exp(-energy + rowmin) with accumulated row-sum, then reciprocal
    multiply.  (softmax(max-e) == exp(rowmin-e)/sum.)
  - The residual is folded into the weights: W' = alpha*softmax + I, so
    the second matmul computes y = W' @ v directly and phase D needs only
    PSUM->SBUF copies (alternating ScalarE/VectorE) before the output
    DMAs.  D runs m-major so its first chunk depends only on h=0's
    softmax; first/last chunks stream per 512 KB j-tile, the rest per
    1 MB.  PSUM pools are scoped per phase: B/C get 6 transpose banks,
    D gets 4 output banks.
"""

from contextlib import ExitStack

import numpy as np

import concourse.bass as bass
import concourse.mybir as mybir
import concourse.tile as tile
from concourse import bacc
from concourse.bass_utils import run_bass_kernel_spmd
from concourse.masks import make_identity

B, C, HH, WW = 8, 256, 128, 128
HW = HH * WW            # 16384
P = 128
H = C // P              # 2 channel chunks
KT = HW // P            # 128 contraction tiles for energy
S_CHUNK = 2048          # columns per streaming DMA chunk (2 MB)
N_CHUNKS = HW // S_CHUNK
S_TILE = 512            # second-matmul moving free dim (1 PSUM bank)
J_PER_CHUNK = S_CHUNK // S_TILE

F32 = mybir.dt.float32
F32R = mybir.dt.float32r


def emit(nc, tc, alpha, ident_r, v_sb, x_v, y_v, phases="ABCD"):
    """One full per-core pass (phases A-D). Pools are scoped inside.

    phases: subset of "ABCD" — used only by the phase-bisect timing
    harness; correctness requires all four.
    """
    do_a = "A" in phases
    do_b = "B" in phases
    do_c = "C" in phases
    do_d = "D" in phases
    # ---- Phase A: stream x into SBUF.  The PE outruns the DMA early on
    # (8.7 us/chunk of transpose+matmul vs 5.7 us/chunk of DMA), so the
    # first chunk is quartered and the second halved to cut PE starvation.
    if do_a:
        for q in range(4):
            sl = slice(q * (S_CHUNK // 4), (q + 1) * (S_CHUNK // 4))
            nc.sync.dma_start(out=v_sb[:, :, sl], in_=x_v[:, :, sl])
        for ck in range(2, 2 * N_CHUNKS):
            sl = slice(ck * (S_CHUNK // 2), (ck + 1) * (S_CHUNK // 2))
            nc.sync.dma_start(out=v_sb[:, :, sl], in_=x_v[:, :, sl])

    with ExitStack() as wctx:
        w_pool = wctx.enter_context(tc.tile_pool(name="w", bufs=1))
        w_sb = [w_pool.tile([P, C], F32R, name=f"w{h}") for h in range(H)]
        wt_sb = [w_pool.tile([P, C], F32R, name=f"wt{g}") for g in range(H)]

        with ExitStack() as bctx:
            vt_pool = bctx.enter_context(tc.tile_pool(name="vt", bufs=8))
            stats = bctx.enter_context(tc.tile_pool(name="stats", bufs=1))
            psum_e = bctx.enter_context(
                tc.tile_pool(name="psum_e", bufs=1, space="PSUM"))
            psum_t = bctx.enter_context(
                tc.tile_pool(name="psum_t", bufs=6, space="PSUM"))

            # ---- Phase B: energy = v @ v.T (two PSUM banks), with the
            # transpose+copy pipelined one k ahead of the matmuls.
            e_ps = [psum_e.tile([P, C], F32, name=f"energy{h}")[:]
                    for h in range(H)]

            def make_vt(k):
                ksl = slice(k * P, (k + 1) * P)
                vt = vt_pool.tile([P, C], F32R, name="vt")
                tp = psum_t.tile([P, C], F32R, name="tp")
                for h in range(H):
                    nc.tensor.transpose(
                        tp[:, h * P:(h + 1) * P], v_sb[:, h, ksl], ident_r[:]
                    )
                if k % 2 == 0:
                    nc.scalar.copy(vt[:], tp[:])
                else:
                    nc.vector.tensor_copy(vt[:], tp[:])
                return vt

            # Two k-tiles per pipeline step: their 4 matmuls run
            # back-to-back so the self-loading weight fetch of each next
            # matmul pulls ahead of the running one (PE reorder window).
            if do_b:
                vts = [make_vt(0), make_vt(1)]
                for k0 in range(0, KT, 2):
                    for kn in (k0 + 2, k0 + 3):
                        if kn < KT:
                            vts.append(make_vt(kn))
                    for k in (k0, k0 + 1):
                        vt_r = vts.pop(0)[:]
                        for h in range(H):
                            nc.tensor.matmul(
                                e_ps[h],
                                lhsT=vt_r[:, h * P:(h + 1) * P],
                                rhs=vt_r,
                                start=(k == 0),
                                stop=(k == KT - 1),
                            )

            # ---- Phase C: row softmax of (max - e) == exp(min - e)/sum,
            # then W' = alpha*softmax + I so phase D's matmul computes
            # alpha*(w@v) + v directly and needs no elementwise pass.
            # Each h's wT transposes follow immediately so everything phase
            # D's m=0 needs is ahead of h=1 work in engine order.
            # wT[g][p, h*128+q] = W'[h][q, g*128+p] for the second matmul.
            for h in range(H if do_c else 0):
                mn = stats.tile([P, 1], F32, name=f"mn{h}")
                sm = stats.tile([P, 1], F32, name=f"sm{h}")
                rc = stats.tile([P, 1], F32, name=f"rc{h}")
                nc.vector.tensor_reduce(
                    mn[:], e_ps[h], axis=mybir.AxisListType.X,
                    op=mybir.AluOpType.min
                )
                nc.scalar.activation(
                    w_sb[h][:], e_ps[h], mybir.ActivationFunctionType.Exp,
                    bias=mn[:], scale=-1.0, accum_out=sm[:],
                )
                nc.vector.reciprocal(rc[:], sm[:])
                nc.vector.tensor_scalar_mul(rc[:], rc[:], alpha)
                nc.vector.tensor_scalar_mul(w_sb[h][:], w_sb[h][:], rc[:])
                nc.vector.tensor_tensor(
                    out=w_sb[h][:, h * P:(h + 1) * P],
                    in0=w_sb[h][:, h * P:(h + 1) * P],
                    in1=ident_r[:],
                    op=mybir.AluOpType.add,
                )
                for g in range(H):
                    tp2 = psum_t.tile([P, C], F32R, name="tp2", tag="tp")
                    nc.tensor.transpose(
                        tp2[:, 0:P], w_sb[h][:, g * P:(g + 1) * P], ident_r[:]
                    )
                    if g == 0:
                        nc.vector.tensor_copy(
                            wt_sb[g][:, h * P:(h + 1) * P], tp2[:, 0:P])
                    else:
                        nc.scalar.copy(
                            wt_sb[g][:, h * P:(h + 1) * P], tp2[:, 0:P])

        # ---- Phase D: y = W' @ v (residual and alpha folded into W').
        # jt-major: both m-blocks of a j-range are computed back to back so
        # v_sb columns are released left to right — the next loop
        # iteration's input DMA (which overlaps this iteration's tail; the
        # For_i barrier syncs engines, not in-flight DMA rings) can refill
        # v_sb behind us.  Each 256 KB j-tile is copied PSUM->SBUF
        # (VectorE:ScalarE 2:1 — GPSIMD cannot read PSUM, and one engine
        # alone would pace the output stream) and fired as its own DMA.
        with ExitStack() as dctx:
            out_pool = dctx.enter_context(tc.tile_pool(name="out", bufs=12))
            psum_y = dctx.enter_context(
                tc.tile_pool(name="psum_y", bufs=6, space="PSUM"))
            n_cp = 0
            for jt in range(HW // S_TILE if do_d else 0):
                j0 = jt * S_TILE
                jsl = slice(j0, j0 + S_TILE)
                for m in range(H):
                    yp = psum_y.tile([P, S_TILE], F32, name="yp")
                    for g in range(H):
                        nc.tensor.matmul(
                            yp[:],
                            lhsT=wt_sb[g][:][:, m * P:(m + 1) * P],
                            rhs=v_sb[:][:, g, jsl],
                            start=(g == 0),
                            stop=(g == H - 1),
                        )
                    ost = out_pool.tile([P, S_TILE], F32, name="ost")
                    if n_cp % 3 == 2:
                        nc.scalar.copy(ost[:], yp[:])
                    else:
                        nc.vector.tensor_copy(ost[:], yp[:])
                    n_cp += 1
                    nc.sync.dma_start(out=y_v[:, m, jsl], in_=ost[:])


def _build(alpha: float) -> bass.Bass:
    # Bacc (not plain Bass): its compile() legalizes semaphore waits into
    # EventSemaphore instructions — hardware allows only 1 wait per
    # instruction and Tile freely emits more.
    nc = bacc.Bacc("TRN2", target_bir_lowering=False)
    # x is declared float32r (same 32-bit layout as fp32 at rest) so the DMA,
    # the PE transposes, and both matmuls form a consistent f32r chain for
    # the BIR verifier; the PE truncates to FP22 on read either way.
    x = nc.dram_tensor("x", [C, HW], F32R, kind="ExternalInput")
    y = nc.dram_tensor("y", [C, HW], F32, kind="ExternalOutput")
    x_v = x.rearrange("(h p) s -> p h s", p=P)
    y_v = y.rearrange("(h p) s -> p h s", p=P)

    with tile.TileContext(nc) as tc, ExitStack() as ctx:
        singles = ctx.enter_context(tc.tile_pool(name="singles", bufs=1))
        ident = singles.tile([P, P], F32, name="ident")
        make_identity(nc, ident)
        ident_r = singles.tile([P, P], F32R, name="ident_r")
        nc.vector.tensor_copy(ident_r[:], ident[:])
        # Whole v resident: 128 KB per partition.
        v_sb = singles.tile([P, H, HW], F32R, name="v_sb")
        emit(nc, tc, alpha, ident_r, v_sb, x_v, y_v)
    nc.compile()
    return nc


def kernel(x: np.ndarray, alpha: np.ndarray, **_kw) -> np.ndarray:
    assert x.shape == (B, C, HH, WW) and x.dtype == np.float32
    xs = np.ascontiguousarray(x.reshape(B, C, HW)).astype(np.float32, copy=False)
    nc = _build(float(np.asarray(alpha).reshape(-1)[0]))
    in_maps = [{"x": xs[b]} for b in range(B)]
    res = run_bass_kernel_spmd(nc, in_maps, core_ids=list(range(B)))
    out = np.stack([np.asarray(r["y"]) for r in res.results])
    return out.reshape(B, C, HH, WW).astype(np.float32, copy=False)

